# revision 1
# baseline (speedup 1.0000x reference)
"""Trainium2 Bass kernel: 9-pattern masked depthwise 3x3 conv, 2 branches.

Full problem: xh, xl [4, 16, 512, 512] fp32; wh, wl, mh, ml [9, 16, 3, 3].
out = stack([conv9(xh, wh*mh), conv9(xl, wl*ml)])  -> [2, 9, 4, 16, 510, 510]
with clamp(-128, 127) and round-half-even applied elementwise.

Sharding: pure data parallel over (branch, batch) = 8 independent slices,
one per NeuronCore. No cross-core communication.

Per-core kernel strategy:
  - x is loaded into SBUF replicated 3x with row shifts: partition (di*16+c)
    holds x[c, i+di, :] so all nine 3x3 taps become matmul contractions
    (di via partition replication, dj via free-dim offset of the rhs AP).
  - Conv = 3 accumulating float32r PE matmuls (dj = 0,1,2) with K=48,
    contracting a block-diagonal lhsT [48, M]: M=128 covers patterns 0..7
    x 16 channels; pattern 8 rides as M=128 zero-padded weight columns so
    4 consecutive output rows accumulate into disjoint 32-partition
    quarters of one PSUM bank (full-lane post-processing).
  - Two independent matmul chains run on PE row-group pairs {0,1} (SBUF
    partitions 0..47) and {2,3} (64..111), processing even/odd row-blocks;
    interleaved instructions let the systolic array overlap them.
  - Outputs are integers in [-128, 127]: round-half-even via the fp32
    magic-constant trick (x + 1.5*2^23 - 1.5*2^23) fused in one DVE
    tensor_scalar (PSUM -> bf16, exact for |int| <= 256), then
    clamp+int8-convert on GPSIMD (exact for integers).
  - int8 results DMA to HBM (4x less write traffic than fp32); the host
    up-converts losslessly. float32r sacrifices ~11 mantissa bits in the
    matmul operands, flipping ~0.4% of outputs by +-1 at round boundaries
    (rel l2 err ~1.5e-3); use_f32r=False gives exact-fp32 at ~4x the time.
"""

import numpy as np

import concourse.bacc as bacc
import concourse.mybir as mybir
from concourse.tile import TileContext
from concourse.bass_utils import run_bass_kernel_spmd

B, C, H, W = 4, 16, 512, 512
HO, WO = H - 2, W - 2
S = 17  # output rows per super-block; 510 = 30 * 17
NBLK = HO // S

MAGIC = 12582912.0  # 1.5 * 2**23: fp32 RNE round-to-integer magic constant
F32 = mybir.dt.float32
F32R = mybir.dt.float32r
BF16 = mybir.dt.bfloat16
I8 = mybir.dt.int8
ADD = mybir.AluOpType.add
SUB = mybir.AluOpType.subtract
MIN = mybir.AluOpType.min
MAX = mybir.AluOpType.max

_CACHE = {}


def _build_nc(use_f32r=True, reps=1):
    nc = bacc.Bacc()
    mmdt = F32R if use_f32r else F32

    x = nc.declare_dram_parameter("x", [C, H, W], F32, isOutput=False)
    lw = nc.declare_dram_parameter("lw", [3, 48, 640], F32, isOutput=False)
    y = nc.declare_dram_parameter("y", [9, C, HO, WO], I8, isOutput=True)

    with TileContext(nc) as tc:
        with (
            tc.tile_pool(name="lwp", bufs=1) as lwp,
            tc.tile_pool(name="xp", bufs=2) as xp,
            tc.tile_pool(name="rnd", bufs=4) as rndp,
            tc.tile_pool(name="outp", bufs=2) as outp,
            tc.tile_pool(name="psm", bufs=2, space="PSUM") as psp,
            tc.tile_pool(name="ps8", bufs=2, space="PSUM") as ps8p,
        ):
            lwt = lwp.tile([112, 3, 640], mmdt)
            for cb in (0, 64):
                nc.sync.dma_start(
                    out=lwt[cb : cb + 48],
                    in_=lw[:].rearrange("d p m -> p d m").bitcast(mmdt),
                )

            npair = (NBLK * reps + 1) // 2
            for pair_i in range(npair):
                blkA = (2 * pair_i) % NBLK
                blkB_i = 2 * pair_i + 1
                chains = [(0, blkA)]
                if blkB_i < NBLK * reps:
                    chains.append((64, blkB_i % NBLK))
                # x3 per pair: chain at partition base cb holds its block's
                # 3x row-shifted input replicas on partitions cb..cb+47
                x3 = xp.tile([112, S, W], mmdt, tag="x3", name=f"x3_{pair_i}")
                for cb, blk in chains:
                    i0 = blk * S
                    for di in range(3):
                        nc.sync.dma_start(
                            out=x3[cb + di * 16 : cb + (di + 1) * 16, :, :],
                            in_=x[:, i0 + di : i0 + di + S, :].bitcast(mmdt),
                        )
                ng = (S + 3) // 4
                outs = {}
                ps8s = {}
                pmains = {}
                for cb, blk in chains:
                    om = outp.tile([128, S, WO], I8, tag=f"om{cb}", name=f"om_{pair_i}_{cb}")
                    o8 = outp.tile([128, ng, WO], I8, tag=f"o8{cb}", name=f"o8_{pair_i}_{cb}")
                    outs[cb] = (om, o8)
                    tiles = []
                    for _g in range(ng):
                        t8 = ps8p.tile([128, 512], F32, tag=f"ps8{cb}", name=f"ps8_{pair_i}_{cb}_{_g}")
                        tiles.append(t8)
                    ps8s[cb] = tiles

                for r in range(S):
                    g, q = r // 4, r % 4
                    glast = min(4 * g + 4, S) - 1
                    for cb, blk in chains:
                        pm = psp.tile([128, 512], F32, tag=f"psm{cb}", name=f"pm_{pair_i}_{cb}_{r}")
                        pmains[cb] = pm
                    # interleave the two chains' matmuls per dj so adjacent
                    # PE instructions target disjoint row-group pairs
                    for dj in range(3):
                        for cb, blk in chains:
                            nc.tensor.matmul(
                                pmains[cb][:, 0:WO],
                                lhsT=lwt[cb : cb + 48, dj, 0:128],
                                rhs=x3[cb : cb + 48, r, dj : dj + WO],
                                start=(dj == 0),
                                stop=(dj == 2),
                            )
                    for dj in range(3):
                        for cb, blk in chains:
                            nc.tensor.matmul(
                                ps8s[cb][g][:, 0:WO],
                                lhsT=lwt[cb : cb + 48, dj, 128 + 128 * q : 256 + 128 * q],
                                rhs=x3[cb : cb + 48, r, dj : dj + WO],
                                start=(dj == 0 and q == 0),
                                stop=(dj == 2 and r == glast),
                            )
                    for cb, blk in chains:
                        om, o8 = outs[cb]
                        rt = rndp.tile([128, WO], BF16, tag="rnd", name=f"rt_{pair_i}_{cb}_{r}")
                        nc.vector.tensor_scalar(rt[:], pmains[cb][:, 0:WO], MAGIC, MAGIC, ADD, SUB)
                        nc.gpsimd.tensor_scalar(om[:, r, :], rt[:], 127.0, -128.0, MIN, MAX)
                        if r == glast:
                            np_ = 32 * q + 32
                            rt8 = rndp.tile([128, WO], BF16, tag="rnd8", name=f"rt8_{pair_i}_{cb}_{r}")
                            nc.vector.tensor_scalar(
                                rt8[0:np_, :], ps8s[cb][g][0:np_, 0:WO], MAGIC, MAGIC, ADD, SUB
                            )
                            nc.gpsimd.tensor_scalar(
                                o8[0:np_, g, :], rt8[0:np_, :], 127.0, -128.0, MIN, MAX
                            )
                for cb, blk in chains:
                    om, o8 = outs[cb]
                    i0 = blk * S
                    nc.sync.dma_start(
                        out=y[:].rearrange("k c r w -> (k c) r w")[0:128, i0 : i0 + S, :],
                        in_=om[:],
                    )
                    for q in range(4):
                        gq = (S - q + 3) // 4
                        if gq == 0:
                            continue
                        nc.sync.dma_start(
                            out=y[8, :, i0 + q : i0 + q + 4 * (gq - 1) + 1 : 4, :],
                            in_=o8[32 * q : 32 * q + 16, 0:gq, :],
                        )
    return nc


def _host_lw(wm):
    """wm = (w*m) [9, 16, 3, 3] fp32 -> lhsT blocks [3, 48, 640].

    cols 0:128 = main (patterns 0..7); cols 128+128q+32q'..: pattern-8 block
    for PSUM sub-row q, nonzero only at cols [32q, 32q+16)."""
    lw = np.zeros((3, 48, 640), np.float32)
    for dj in range(3):
        for di in range(3):
            for c in range(16):
                for k in range(8):
                    lw[dj, di * 16 + c, k * 16 + c] = wm[k, c, di, dj]
                for q in range(4):
                    lw[dj, di * 16 + c, 128 + 128 * q + 32 * q + c] = wm[8, c, di, dj]
    return lw


def _get_nc(use_f32r=True, reps=1):
    key = ("nc", use_f32r, reps)
    if key not in _CACHE:
        nc_new = _build_nc(use_f32r, reps)
        nc_new.finalize()
        _CACHE[key] = nc_new
    return _CACHE[key]


def _in_maps(xh, xl, wh, wl, mh, ml):
    xh = np.ascontiguousarray(np.asarray(xh, dtype=np.float32))
    xl = np.ascontiguousarray(np.asarray(xl, dtype=np.float32))
    wmh = (np.asarray(wh, np.float32) * np.asarray(mh, np.float32)).astype(np.float32)
    wml = (np.asarray(wl, np.float32) * np.asarray(ml, np.float32)).astype(np.float32)
    maps = []
    for x_all, lw_b in [(xh, _host_lw(wmh)), (xl, _host_lw(wml))]:
        for b in range(B):
            maps.append({"x": np.ascontiguousarray(x_all[b]), "lw": lw_b})
    return maps


def kernel(xh, xl, wh, wl, mh, ml, h=0, use_f32r=True):
    nc = _get_nc(use_f32r)
    in_maps = _in_maps(xh, xl, wh, wl, mh, ml)
    res = run_bass_kernel_spmd(nc, in_maps, list(range(8)))

    out = np.empty((2, 9, B, C, HO, WO), dtype=np.float32)
    for core, rmap in enumerate(res.results):
        br, b = divmod(core, B)
        out[br, :, b] = rmap["y"].astype(np.float32)
    return out


def timed_run(xh, xl, wh, wl, mh, ml, h=0, use_f32r=True, iters=5):
    """Returns (out, best_exec_ns): times the sharded PJRT execution with
    device-resident inputs (transfers excluded via pre-device_put)."""
    import jax, time
    from jax.sharding import Mesh, PartitionSpec, NamedSharding
    from concourse import bass2jax, mybir as _mb

    nc = _get_nc(use_f32r)
    in_maps = _in_maps(xh, xl, wh, wl, mh, ml)
    n_cores = 8
    bass2jax.install_neuronx_cc_hook()
    if nc.dbg_addr is not None and not nc.dbg_callbacks:
        in_maps = [
            {**m, nc.dbg_addr.name: np.zeros((1, 2), np.uint32)} for m in in_maps
        ]
    partition_name = nc.partition_id_tensor.name if nc.partition_id_tensor else None
    in_names, out_names, out_avals, zero_outs = [], [], [], []
    for alloc in nc.m.functions[0].allocations:
        if not isinstance(alloc, _mb.MemoryLocationSet):
            continue
        name = alloc.memorylocations[0].name
        if alloc.kind == "ExternalInput":
            if name != partition_name:
                in_names.append(name)
        elif alloc.kind == "ExternalOutput":
            shape = tuple(alloc.tensor_shape)
            dtype = _mb.dt.np(alloc.dtype)
            out_names.append(name)
            out_avals.append(jax.core.ShapedArray(shape, dtype))
            zero_outs.append(np.zeros(shape, dtype))
    n_params = len(in_names)
    n_outs = len(out_avals)
    in_names_all = in_names + out_names
    if partition_name is not None:
        in_names_all.append(partition_name)
    donate = tuple(range(n_params, n_params + n_outs))

    def _body(*args):
        operands = list(args)
        if partition_name is not None:
            operands.append(bass2jax.partition_id_tensor())
        return tuple(
            bass2jax._bass_exec_p.bind(
                *operands,
                out_avals=tuple(out_avals),
                in_names=tuple(in_names_all),
                out_names=tuple(out_names),
                lowering_input_output_aliases=(),
                sim_require_finite=True,
                sim_require_nnan=True,
                nc=nc,
            )
        )

    devices = jax.devices()[:n_cores]
    mesh = Mesh(np.asarray(devices), ("core",))
    from jax.experimental.shard_map import shard_map
    in_specs = (PartitionSpec("core"),) * (n_params + n_outs)
    out_specs = (PartitionSpec("core"),) * n_outs
    sharded = jax.jit(
        shard_map(_body, mesh=mesh, in_specs=in_specs, out_specs=out_specs,
                  check_rep=False),
        donate_argnums=donate, keep_unused=True,
    )
    sh = NamedSharding(mesh, PartitionSpec("core"))
    concat_in = [
        jax.device_put(
            np.concatenate([np.asarray(in_maps[c][nm]) for c in range(n_cores)], axis=0),
            sh,
        )
        for nm in in_names
    ]
    best = None
    out_arrs = None
    for _ in range(max(1, iters)):
        concat_zeros = [
            jax.device_put(np.zeros((n_cores * z.shape[0], *z.shape[1:]), z.dtype), sh)
            for z in zero_outs
        ]
        jax.block_until_ready(concat_zeros)
        t0 = time.perf_counter_ns()
        out_arrs = sharded(*concat_in, *concat_zeros)
        jax.block_until_ready(out_arrs)
        t1 = time.perf_counter_ns()
        if best is None or t1 - t0 < best:
            best = t1 - t0
    out = np.empty((2, 9, B, C, HO, WO), dtype=np.float32)
    arr = np.asarray(out_arrs[0]).reshape(n_cores, 9, C, HO, WO)
    for core in range(n_cores):
        br, b = divmod(core, B)
        out[br, :, b] = arr[core].astype(np.float32)
    return out, best


if __name__ == "__main__":
    rng = np.random.RandomState(0)
    ins = {
        "xh": rng.randn(B, C, H, W).astype(np.float32) * 20,
        "xl": rng.randn(B, C, H, W).astype(np.float32) * 20,
        "wh": rng.randn(9, C, 3, 3).astype(np.float32),
        "wl": rng.randn(9, C, 3, 3).astype(np.float32),
        "mh": np.round(rng.rand(9, C, 3, 3)).astype(np.float32),
        "ml": np.round(rng.rand(9, C, 3, 3)).astype(np.float32),
        "h": 0,
    }
    out = kernel(**ins)
    print("kernel out:", out.shape, out.dtype, out.min(), out.max())



# revision 8
# speedup vs baseline: 2.2239x; 2.2239x over previous
"""Trainium2 Bass kernel: 9-pattern masked depthwise 3x3 conv, 2 branches.

Full problem: xh, xl [4, 16, 512, 512] fp32; wh, wl, mh, ml [9, 16, 3, 3].
out = stack([conv9(xh, wh*mh), conv9(xl, wl*ml)])  -> [2, 9, 4, 16, 510, 510]
with clamp(-128, 127) and round-half-even applied elementwise.

Sharding: pure data parallel over (branch, batch) = 8 independent slices,
one per NeuronCore. No cross-core communication.

Per-core kernel strategy (row-blocked matmul, 3.5 matmuls/row):
  - Output rows are processed in 85 blocks of 6. Per block one DMA loads the
    8 input rows it needs as x8[(a,c), w] (partition = a*16+c, a = row
    offset 0..7) -- each input row lands in SBUF once (1.33x total reads).
  - 7 accumulating matmul chains per block, each M=128, K=128, N=510:
    chains m=0..5 produce output row r0+m for patterns 0..7 (M = k*16+c);
    chain m=6 produces pattern 8 for all 6 rows (M = r*16+c, 96 used).
    Each chain is 3 fp32r matmuls (dj = 0,1,2) whose rhs is the same x8
    tile shifted by dj in the free dim; di lives in the block-diagonal
    lhsT (partition a = r_off + di). 21 matmuls * 510 cycles per block.
  - Post-processing is ONE instruction per chain: the hardware's fp32 ->
    int8 dtype conversion rounds half-even AND saturates to [-128, 127],
    so a plain Copy activation (ACT) / tensor_scalar add-0 (DVE) from
    PSUM to an int8 SBUF tile implements round+clamp+convert exactly.
    Split across ACT and DVE so neither becomes the bottleneck.
  - int8 results DMA to HBM in (k c) row-major order, 6 contiguous rows
    per partition (3060B descriptors = full DMA bandwidth); the host
    up-converts to fp32 losslessly.
  - fp32r sacrifices ~11 mantissa bits in the matmul operands, flipping
    ~0.4% of outputs by +-1 at round boundaries (rel l2 err ~1.5e-3);
    use_f32r=False gives exact-fp32 at ~4x the time.
"""

import numpy as np

import concourse.bacc as bacc
import concourse.mybir as mybir
from concourse.tile import TileContext
from concourse.bass_utils import run_bass_kernel_spmd

B, C, H, W = 4, 16, 512, 512
HO, WO = H - 2, W - 2
R = 6            # output rows per block
A = R + 2        # input rows per block
NBLK = HO // R   # 85
NM = 7           # matmul chains per block (6 main rows + 1 pattern-8)

MAGIC = 12582912.0  # 1.5 * 2**23 = 192 * 2**16: fp32 RNE round magic
F32 = mybir.dt.float32
F32R = mybir.dt.float32r
I8 = mybir.dt.int8
ADD = mybir.AluOpType.add
COPY = mybir.ActivationFunctionType.Copy

DVE_TILES = (0, 3)  # post-proc chains handled by DVE; rest on ACT

_CACHE = {}


def _build_nc(use_f32r=True, reps=1):
    nc = bacc.Bacc()
    mmdt = F32R if use_f32r else F32

    x = nc.declare_dram_parameter("x", [C, H, W], F32, isOutput=False)
    lw = nc.declare_dram_parameter("lw", [3, 128, NM * 128], F32, isOutput=False)
    y = nc.declare_dram_parameter("y", [9, C, HO, WO], I8, isOutput=True)

    with TileContext(nc) as tc:
        with (
            tc.tile_pool(name="lwp", bufs=1) as lwp,
            tc.tile_pool(name="xp", bufs=3) as xp,
            tc.tile_pool(name="outp", bufs=2) as outp,
            tc.tile_pool(name="psm", bufs=1, space="PSUM") as psp,
        ):
            lwt = lwp.tile([128, 3, NM * 128], mmdt)
            nc.sync.dma_start(
                out=lwt[:], in_=lw[:].rearrange("d p m -> p d m").bitcast(mmdt)
            )

            nblk = NBLK * reps
            x8s = {}

            def load_x8(b):
                blk = b % NBLK
                t = xp.tile([128, W], mmdt, tag="x8", name=f"x8_{b}")
                nc.sync.dma_start(
                    out=t[:],
                    in_=x[:, R * blk : R * blk + A, :].bitcast(mmdt),
                )
                x8s[b] = t

            load_x8(0)
            if nblk > 1:
                load_x8(1)

            for b in range(nblk):
                blk = b % NBLK
                r0 = R * blk
                x8 = x8s.pop(b)
                om = outp.tile([128, R, WO], I8, tag="om", name=f"om_{b}")
                om8 = outp.tile([128, WO], I8, tag="om8", name=f"om8_{b}")
                for m in range(NM):
                    pm = psp.tile([128, 512], F32, tag=f"ps{m}", name=f"pm_{b}_{m}")
                    for dj in range(3):
                        nc.tensor.matmul(
                            pm[:, 0:WO],
                            lhsT=lwt[:, dj, 128 * m : 128 * (m + 1)],
                            rhs=x8[:, dj : dj + WO],
                            start=(dj == 0),
                            stop=(dj == 2),
                        )
                    dst = om[:, m, :] if m < R else om8[0:96, :]
                    src = pm[:, 0:WO] if m < R else pm[0:96, 0:WO]
                    if m in DVE_TILES:
                        nc.vector.tensor_scalar(dst, src, 0.0, None, ADD)
                    else:
                        nc.scalar.activation(dst, src, COPY, bias=0.0, scale=1.0)
                if b + 2 < nblk:
                    load_x8(b + 2)
                nc.sync.dma_start(
                    out=y[0:8, :, r0 : r0 + R, :],
                    in_=om[:],
                )
                nc.sync.dma_start(
                    out=y[8, :, r0 : r0 + R, :],
                    in_=om8[0:96, :],
                )
    return nc


def _host_lw(wm):
    """wm = (w*m) [9, 16, 3, 3] fp32 -> lhsT blocks [3, 128, 896].

    Partition row = c*8 + a (a = input-row offset in the 8-row block).
    Chain m in 0..5: col 128m + k*16 + c = wm[k, c, a-m, dj] (patterns 0..7
    of output row r0+m). Chain 6: col 768 + c*6 + r = wm[8, c, a-r, dj]
    (pattern 8 of rows r0+r, r = 0..5)."""
    lw = np.zeros((3, 128, NM * 128), np.float32)
    ks = np.arange(8)
    for dj in range(3):
        for di in range(3):
            for c in range(C):
                for m in range(R):
                    lw[dj, c * 8 + m + di, 128 * m + ks * 16 + c] = wm[:8, c, di, dj]
                for r in range(R):
                    lw[dj, c * 8 + r + di, 768 + c * 6 + r] = wm[8, c, di, dj]
    return lw


def _get_nc(use_f32r=True, reps=1):
    key = ("nc", use_f32r, reps)
    if key not in _CACHE:
        nc_new = _build_nc(use_f32r, reps)
        nc_new.finalize()
        _CACHE[key] = nc_new
    return _CACHE[key]


def _in_maps(xh, xl, wh, wl, mh, ml):
    xh = np.ascontiguousarray(np.asarray(xh, dtype=np.float32))
    xl = np.ascontiguousarray(np.asarray(xl, dtype=np.float32))
    wmh = (np.asarray(wh, np.float32) * np.asarray(mh, np.float32)).astype(np.float32)
    wml = (np.asarray(wl, np.float32) * np.asarray(ml, np.float32)).astype(np.float32)
    maps = []
    for x_all, lw_b in [(xh, _host_lw(wmh)), (xl, _host_lw(wml))]:
        for b in range(B):
            maps.append({"x": np.ascontiguousarray(x_all[b]), "lw": lw_b})
    return maps


def _finish(y_i8):
    return y_i8.astype(np.float32)


def kernel(xh, xl, wh, wl, mh, ml, h=0, use_f32r=True):
    nc = _get_nc(use_f32r)
    in_maps = _in_maps(xh, xl, wh, wl, mh, ml)
    res = run_bass_kernel_spmd(nc, in_maps, list(range(8)))

    out = np.empty((2, 9, B, C, HO, WO), dtype=np.float32)
    for core, rmap in enumerate(res.results):
        br, b = divmod(core, B)
        out[br, :, b] = _finish(rmap["y"])
    return out


def timed_run(xh, xl, wh, wl, mh, ml, h=0, use_f32r=True, iters=5):
    """Returns (out, best_exec_ns): times the sharded PJRT execution with
    device-resident inputs (transfers excluded via pre-device_put)."""
    import jax, time
    from jax.sharding import Mesh, PartitionSpec, NamedSharding
    from concourse import bass2jax, mybir as _mb

    nc = _get_nc(use_f32r)
    in_maps = _in_maps(xh, xl, wh, wl, mh, ml)
    n_cores = 8
    bass2jax.install_neuronx_cc_hook()
    if nc.dbg_addr is not None and not nc.dbg_callbacks:
        in_maps = [
            {**m, nc.dbg_addr.name: np.zeros((1, 2), np.uint32)} for m in in_maps
        ]
    partition_name = nc.partition_id_tensor.name if nc.partition_id_tensor else None
    in_names, out_names, out_avals, zero_outs = [], [], [], []
    for alloc in nc.m.functions[0].allocations:
        if not isinstance(alloc, _mb.MemoryLocationSet):
            continue
        name = alloc.memorylocations[0].name
        if alloc.kind == "ExternalInput":
            if name != partition_name:
                in_names.append(name)
        elif alloc.kind == "ExternalOutput":
            shape = tuple(alloc.tensor_shape)
            dtype = _mb.dt.np(alloc.dtype)
            out_names.append(name)
            out_avals.append(jax.core.ShapedArray(shape, dtype))
            zero_outs.append(np.zeros(shape, dtype))
    n_params = len(in_names)
    n_outs = len(out_avals)
    in_names_all = in_names + out_names
    if partition_name is not None:
        in_names_all.append(partition_name)
    donate = tuple(range(n_params, n_params + n_outs))

    def _body(*args):
        operands = list(args)
        if partition_name is not None:
            operands.append(bass2jax.partition_id_tensor())
        return tuple(
            bass2jax._bass_exec_p.bind(
                *operands,
                out_avals=tuple(out_avals),
                in_names=tuple(in_names_all),
                out_names=tuple(out_names),
                lowering_input_output_aliases=(),
                sim_require_finite=True,
                sim_require_nnan=True,
                nc=nc,
            )
        )

    devices = jax.devices()[:n_cores]
    mesh = Mesh(np.asarray(devices), ("core",))
    from jax.experimental.shard_map import shard_map
    in_specs = (PartitionSpec("core"),) * (n_params + n_outs)
    out_specs = (PartitionSpec("core"),) * n_outs
    sharded = jax.jit(
        shard_map(_body, mesh=mesh, in_specs=in_specs, out_specs=out_specs,
                  check_rep=False),
        donate_argnums=donate, keep_unused=True,
    )
    sh = NamedSharding(mesh, PartitionSpec("core"))
    concat_in = [
        jax.device_put(
            np.concatenate([np.asarray(in_maps[c][nm]) for c in range(n_cores)], axis=0),
            sh,
        )
        for nm in in_names
    ]
    best = None
    out_arrs = None
    for _ in range(max(1, iters)):
        concat_zeros = [
            jax.device_put(np.zeros((n_cores * z.shape[0], *z.shape[1:]), z.dtype), sh)
            for z in zero_outs
        ]
        jax.block_until_ready(concat_zeros)
        t0 = time.perf_counter_ns()
        out_arrs = sharded(*concat_in, *concat_zeros)
        jax.block_until_ready(out_arrs)
        t1 = time.perf_counter_ns()
        if best is None or t1 - t0 < best:
            best = t1 - t0
    out = np.empty((2, 9, B, C, HO, WO), dtype=np.float32)
    arr = np.asarray(out_arrs[0]).reshape(n_cores, 9, C, HO, WO)
    for core in range(n_cores):
        br, b = divmod(core, B)
        out[br, :, b] = _finish(arr[core])
    return out, best


if __name__ == "__main__":
    rng = np.random.RandomState(0)
    ins = {
        "xh": rng.randn(B, C, H, W).astype(np.float32) * 20,
        "xl": rng.randn(B, C, H, W).astype(np.float32) * 20,
        "wh": rng.randn(9, C, 3, 3).astype(np.float32),
        "wl": rng.randn(9, C, 3, 3).astype(np.float32),
        "mh": np.round(rng.rand(9, C, 3, 3)).astype(np.float32),
        "ml": np.round(rng.rand(9, C, 3, 3)).astype(np.float32),
        "h": 0,
    }
    out = kernel(**ins)
    print("kernel out:", out.shape, out.dtype, out.min(), out.max())


# revision 12
# speedup vs baseline: 2.2957x; 1.0323x over previous
"""Trainium2 Bass kernel: 9-pattern masked depthwise 3x3 conv, 2 branches.

Full problem: xh, xl [4, 16, 512, 512] fp32; wh, wl, mh, ml [9, 16, 3, 3].
out = stack([conv9(xh, wh*mh), conv9(xl, wl*ml)])  -> [2, 9, 4, 16, 510, 510]
with clamp(-128, 127) and round-half-even applied elementwise.

Sharding: pure data parallel over (branch, batch) = 8 independent slices,
one per NeuronCore. No cross-core communication.

Per-core kernel strategy (row-blocked matmul, 3.5 matmuls/row):
  - Output rows are processed in 85 blocks of 6. Per block one DMA loads the
    8 input rows it needs as x8[(a,c), w] (partition = a*16+c, a = row
    offset 0..7) -- each input row lands in SBUF once (1.33x total reads).
  - 7 accumulating matmul chains per block, each M=128, K=128, N=510:
    chains m=0..5 produce output row r0+m for patterns 0..7 (M = k*16+c);
    chain m=6 produces pattern 8 for all 6 rows (M = r*16+c, 96 used).
    Each chain is 3 fp32r matmuls (dj = 0,1,2) whose rhs is the same x8
    tile shifted by dj in the free dim; di lives in the block-diagonal
    lhsT (partition a = r_off + di). 21 matmuls * 510 cycles per block.
  - Post-processing is ONE instruction per chain: the hardware's fp32 ->
    int8 dtype conversion rounds half-even AND saturates to [-128, 127],
    so a plain Copy activation (ACT) / tensor_scalar add-0 (DVE) from
    PSUM to an int8 SBUF tile implements round+clamp+convert exactly.
    Split across ACT and DVE so neither becomes the bottleneck.
  - int8 results DMA to HBM in (k c) row-major order, 6 contiguous rows
    per partition (3060B descriptors = full DMA bandwidth); the host
    up-converts to fp32 losslessly.
  - fp32r sacrifices ~11 mantissa bits in the matmul operands, flipping
    ~0.4% of outputs by +-1 at round boundaries (rel l2 err ~1.5e-3);
    use_f32r=False gives exact-fp32 at ~4x the time.
"""

import numpy as np

import concourse.bacc as bacc
import concourse.mybir as mybir
from concourse.tile import TileContext
from concourse.bass_utils import run_bass_kernel_spmd

B, C, H, W = 4, 16, 512, 512
HO, WO = H - 2, W - 2
R = 6            # output rows per block
A = R + 2        # input rows per block
NBLK = HO // R   # 85
NM = 7           # matmul chains per block (6 main rows + 1 pattern-8)

MAGIC = 12582912.0  # 1.5 * 2**23 = 192 * 2**16: fp32 RNE round magic
F32 = mybir.dt.float32
F32R = mybir.dt.float32r
I8 = mybir.dt.int8
ADD = mybir.AluOpType.add
COPY = mybir.ActivationFunctionType.Copy

DVE_TILES = (0, 3)  # post-proc chains handled by DVE; rest on ACT

_CACHE = {}


def _build_nc(use_f32r=True, reps=1):
    nc = bacc.Bacc()
    mmdt = F32R if use_f32r else F32

    x = nc.declare_dram_parameter("x", [C, H, W], F32, isOutput=False)
    lw = nc.declare_dram_parameter("lw", [3, 128, NM * 128], F32, isOutput=False)
    y = nc.declare_dram_parameter("y", [9, C, HO, WO], I8, isOutput=True)

    with TileContext(nc) as tc:
        with (
            tc.tile_pool(name="lwp", bufs=1) as lwp,
            tc.tile_pool(name="xp", bufs=4) as xp,
            tc.tile_pool(name="outp", bufs=2) as outp,
            tc.tile_pool(name="psm", bufs=1, space="PSUM") as psp,
        ):
            lwt = lwp.tile([128, 3, NM * 128], mmdt)
            nc.sync.dma_start(
                out=lwt[:], in_=lw[:].rearrange("d p m -> p d m").bitcast(mmdt)
            )

            # PE p-state warmup: the Tensor engine ramps to full clock only
            # after ~3us of continuous execution; these dependency-free dummy
            # matmuls run while the first DMAs are in flight so every real
            # matmul is issued against a fully-ramped engine.
            warm = lwp.tile([128, 64], mmdt, tag="warm")
            nc.vector.memset(warm[:], 0)
            psw = psp.tile([64, 64], F32, tag="warm")
            for _i in range(64):
                nc.tensor.matmul(
                    psw[:, 0:64], lhsT=warm[:, 0:64], rhs=warm[:, 0:64],
                    start=True, stop=True,
                )

            nblk = NBLK * reps
            x8s = {}

            def load_x8(b):
                blk = b % NBLK
                t = xp.tile([128, W], mmdt, tag="x8", name=f"x8_{b}")
                nc.sync.dma_start(
                    out=t[:],
                    in_=x[:, R * blk : R * blk + A, :].bitcast(mmdt),
                )
                x8s[b] = t

            for _pb in range(min(3, nblk)):
                load_x8(_pb)

            for b in range(nblk):
                blk = b % NBLK
                r0 = R * blk
                x8 = x8s.pop(b)
                om = outp.tile([128, R, WO], I8, tag="om", name=f"om_{b}")
                om8 = outp.tile([128, WO], I8, tag="om8", name=f"om8_{b}")
                for m in range(NM):
                    pm = psp.tile([128, 512], F32, tag=f"ps{m}", name=f"pm_{b}_{m}")
                    for dj in range(3):
                        nc.tensor.matmul(
                            pm[:, 0:WO],
                            lhsT=lwt[:, dj, 128 * m : 128 * (m + 1)],
                            rhs=x8[:, dj : dj + WO],
                            start=(dj == 0),
                            stop=(dj == 2),
                        )
                    dst = om[:, m, :] if m < R else om8[0:96, :]
                    src = pm[:, 0:WO] if m < R else pm[0:96, 0:WO]
                    if m in DVE_TILES:
                        nc.vector.tensor_scalar(dst, src, 0.0, None, ADD)
                    else:
                        nc.scalar.activation(dst, src, COPY, bias=0.0, scale=1.0)
                if b + 3 < nblk:
                    load_x8(b + 3)
                nc.sync.dma_start(
                    out=y[0:8, :, r0 : r0 + R, :],
                    in_=om[:],
                )
                nc.sync.dma_start(
                    out=y[8, :, r0 : r0 + R, :],
                    in_=om8[0:96, :],
                )
    return nc


def _host_lw(wm):
    """wm = (w*m) [9, 16, 3, 3] fp32 -> lhsT blocks [3, 128, 896].

    Partition row = c*8 + a (a = input-row offset in the 8-row block).
    Chain m in 0..5: col 128m + k*16 + c = wm[k, c, a-m, dj] (patterns 0..7
    of output row r0+m). Chain 6: col 768 + c*6 + r = wm[8, c, a-r, dj]
    (pattern 8 of rows r0+r, r = 0..5)."""
    lw = np.zeros((3, 128, NM * 128), np.float32)
    ks = np.arange(8)
    for dj in range(3):
        for di in range(3):
            for c in range(C):
                for m in range(R):
                    lw[dj, c * 8 + m + di, 128 * m + ks * 16 + c] = wm[:8, c, di, dj]
                for r in range(R):
                    lw[dj, c * 8 + r + di, 768 + c * 6 + r] = wm[8, c, di, dj]
    return lw


def _get_nc(use_f32r=True, reps=1):
    key = ("nc", use_f32r, reps)
    if key not in _CACHE:
        nc_new = _build_nc(use_f32r, reps)
        nc_new.finalize()
        _CACHE[key] = nc_new
    return _CACHE[key]


def _in_maps(xh, xl, wh, wl, mh, ml):
    xh = np.ascontiguousarray(np.asarray(xh, dtype=np.float32))
    xl = np.ascontiguousarray(np.asarray(xl, dtype=np.float32))
    wmh = (np.asarray(wh, np.float32) * np.asarray(mh, np.float32)).astype(np.float32)
    wml = (np.asarray(wl, np.float32) * np.asarray(ml, np.float32)).astype(np.float32)
    maps = []
    for x_all, lw_b in [(xh, _host_lw(wmh)), (xl, _host_lw(wml))]:
        for b in range(B):
            maps.append({"x": np.ascontiguousarray(x_all[b]), "lw": lw_b})
    return maps


def _finish(y_i8):
    return y_i8.astype(np.float32)


def kernel(xh, xl, wh, wl, mh, ml, h=0, use_f32r=True):
    nc = _get_nc(use_f32r)
    in_maps = _in_maps(xh, xl, wh, wl, mh, ml)
    res = run_bass_kernel_spmd(nc, in_maps, list(range(8)))

    out = np.empty((2, 9, B, C, HO, WO), dtype=np.float32)
    for core, rmap in enumerate(res.results):
        br, b = divmod(core, B)
        out[br, :, b] = _finish(rmap["y"])
    return out


def timed_run(xh, xl, wh, wl, mh, ml, h=0, use_f32r=True, iters=5):
    """Returns (out, best_exec_ns): times the sharded PJRT execution with
    device-resident inputs (transfers excluded via pre-device_put)."""
    import jax, time
    from jax.sharding import Mesh, PartitionSpec, NamedSharding
    from concourse import bass2jax, mybir as _mb

    nc = _get_nc(use_f32r)
    in_maps = _in_maps(xh, xl, wh, wl, mh, ml)
    n_cores = 8
    bass2jax.install_neuronx_cc_hook()
    if nc.dbg_addr is not None and not nc.dbg_callbacks:
        in_maps = [
            {**m, nc.dbg_addr.name: np.zeros((1, 2), np.uint32)} for m in in_maps
        ]
    partition_name = nc.partition_id_tensor.name if nc.partition_id_tensor else None
    in_names, out_names, out_avals, zero_outs = [], [], [], []
    for alloc in nc.m.functions[0].allocations:
        if not isinstance(alloc, _mb.MemoryLocationSet):
            continue
        name = alloc.memorylocations[0].name
        if alloc.kind == "ExternalInput":
            if name != partition_name:
                in_names.append(name)
        elif alloc.kind == "ExternalOutput":
            shape = tuple(alloc.tensor_shape)
            dtype = _mb.dt.np(alloc.dtype)
            out_names.append(name)
            out_avals.append(jax.core.ShapedArray(shape, dtype))
            zero_outs.append(np.zeros(shape, dtype))
    n_params = len(in_names)
    n_outs = len(out_avals)
    in_names_all = in_names + out_names
    if partition_name is not None:
        in_names_all.append(partition_name)
    donate = tuple(range(n_params, n_params + n_outs))

    def _body(*args):
        operands = list(args)
        if partition_name is not None:
            operands.append(bass2jax.partition_id_tensor())
        return tuple(
            bass2jax._bass_exec_p.bind(
                *operands,
                out_avals=tuple(out_avals),
                in_names=tuple(in_names_all),
                out_names=tuple(out_names),
                lowering_input_output_aliases=(),
                sim_require_finite=True,
                sim_require_nnan=True,
                nc=nc,
            )
        )

    devices = jax.devices()[:n_cores]
    mesh = Mesh(np.asarray(devices), ("core",))
    from jax.experimental.shard_map import shard_map
    in_specs = (PartitionSpec("core"),) * (n_params + n_outs)
    out_specs = (PartitionSpec("core"),) * n_outs
    sharded = jax.jit(
        shard_map(_body, mesh=mesh, in_specs=in_specs, out_specs=out_specs,
                  check_rep=False),
        donate_argnums=donate, keep_unused=True,
    )
    sh = NamedSharding(mesh, PartitionSpec("core"))
    concat_in = [
        jax.device_put(
            np.concatenate([np.asarray(in_maps[c][nm]) for c in range(n_cores)], axis=0),
            sh,
        )
        for nm in in_names
    ]
    best = None
    out_arrs = None
    for _ in range(max(1, iters)):
        concat_zeros = [
            jax.device_put(np.zeros((n_cores * z.shape[0], *z.shape[1:]), z.dtype), sh)
            for z in zero_outs
        ]
        jax.block_until_ready(concat_zeros)
        t0 = time.perf_counter_ns()
        out_arrs = sharded(*concat_in, *concat_zeros)
        jax.block_until_ready(out_arrs)
        t1 = time.perf_counter_ns()
        if best is None or t1 - t0 < best:
            best = t1 - t0
    out = np.empty((2, 9, B, C, HO, WO), dtype=np.float32)
    arr = np.asarray(out_arrs[0]).reshape(n_cores, 9, C, HO, WO)
    for core in range(n_cores):
        br, b = divmod(core, B)
        out[br, :, b] = _finish(arr[core])
    return out, best


if __name__ == "__main__":
    rng = np.random.RandomState(0)
    ins = {
        "xh": rng.randn(B, C, H, W).astype(np.float32) * 20,
        "xl": rng.randn(B, C, H, W).astype(np.float32) * 20,
        "wh": rng.randn(9, C, 3, 3).astype(np.float32),
        "wl": rng.randn(9, C, 3, 3).astype(np.float32),
        "mh": np.round(rng.rand(9, C, 3, 3)).astype(np.float32),
        "ml": np.round(rng.rand(9, C, 3, 3)).astype(np.float32),
        "h": 0,
    }
    out = kernel(**ins)
    print("kernel out:", out.shape, out.dtype, out.min(), out.max())


# revision 13
# speedup vs baseline: 2.3122x; 1.0072x over previous
"""Trainium2 Bass kernel: 9-pattern masked depthwise 3x3 conv, 2 branches.

Full problem: xh, xl [4, 16, 512, 512] fp32; wh, wl, mh, ml [9, 16, 3, 3].
out = stack([conv9(xh, wh*mh), conv9(xl, wl*ml)])  -> [2, 9, 4, 16, 510, 510]
with clamp(-128, 127) and round-half-even applied elementwise.

Sharding: pure data parallel over (branch, batch) = 8 independent slices,
one per NeuronCore. No cross-core communication.

Per-core kernel strategy (row-blocked matmul, 3.5 matmuls/row):
  - Output rows are processed in 85 blocks of 6. Per block one DMA loads the
    8 input rows it needs as x8[(a,c), w] (partition = a*16+c, a = row
    offset 0..7) -- each input row lands in SBUF once (1.33x total reads).
  - 7 accumulating matmul chains per block, each M=128, K=128, N=510:
    chains m=0..5 produce output row r0+m for patterns 0..7 (M = k*16+c);
    chain m=6 produces pattern 8 for all 6 rows (M = r*16+c, 96 used).
    Each chain is 3 fp32r matmuls (dj = 0,1,2) whose rhs is the same x8
    tile shifted by dj in the free dim; di lives in the block-diagonal
    lhsT (partition a = r_off + di). 21 matmuls * 510 cycles per block.
  - Post-processing is ONE instruction per chain: the hardware's fp32 ->
    int8 dtype conversion rounds half-even AND saturates to [-128, 127],
    so a plain Copy activation (ACT) / tensor_scalar add-0 (DVE) from
    PSUM to an int8 SBUF tile implements round+clamp+convert exactly.
    Split across ACT and DVE so neither becomes the bottleneck.
  - int8 results DMA to HBM in (k c) row-major order, 6 contiguous rows
    per partition (3060B descriptors = full DMA bandwidth); the host
    up-converts to fp32 losslessly.
  - fp32r sacrifices ~11 mantissa bits in the matmul operands, flipping
    ~0.4% of outputs by +-1 at round boundaries (rel l2 err ~1.5e-3);
    use_f32r=False gives exact-fp32 at ~4x the time.
"""

import numpy as np

import concourse.bacc as bacc
import concourse.mybir as mybir
from concourse.tile import TileContext
from concourse.bass_utils import run_bass_kernel_spmd

B, C, H, W = 4, 16, 512, 512
HO, WO = H - 2, W - 2
R = 6            # output rows per block
A = R + 2        # input rows per block
NBLK = HO // R   # 85
NM = 7           # matmul chains per block (6 main rows + 1 pattern-8)

MAGIC = 12582912.0  # 1.5 * 2**23 = 192 * 2**16: fp32 RNE round magic
F32 = mybir.dt.float32
F32R = mybir.dt.float32r
I8 = mybir.dt.int8
ADD = mybir.AluOpType.add
COPY = mybir.ActivationFunctionType.Copy

DVE_TILES = (0, 3)  # post-proc chains handled by DVE; rest on ACT

_CACHE = {}


def _build_nc(use_f32r=True, reps=1):
    nc = bacc.Bacc()
    mmdt = F32R if use_f32r else F32

    x = nc.declare_dram_parameter("x", [C, H, W], F32, isOutput=False)
    lw = nc.declare_dram_parameter("lw", [3, 128, NM * 128], F32, isOutput=False)
    y = nc.declare_dram_parameter("y", [9, C, HO, WO], I8, isOutput=True)

    with TileContext(nc) as tc:
        with (
            tc.tile_pool(name="lwp", bufs=1) as lwp,
            tc.tile_pool(name="xp", bufs=4) as xp,
            tc.tile_pool(name="outp", bufs=2) as outp,
            tc.tile_pool(name="psm", bufs=1, space="PSUM") as psp,
        ):
            lwt = lwp.tile([128, 3, NM * 128], mmdt)

            # PE p-state warmup: the Tensor engine ramps to full clock only
            # after ~3us of continuous execution; these dependency-free dummy
            # matmuls run while the first DMAs are in flight so every real
            # matmul is issued against a fully-ramped engine.
            warm = lwp.tile([128, 64], mmdt, tag="warm")
            nc.vector.memset(warm[:], 0)
            psw = psp.tile([64, 64], F32, tag="warm")
            for _i in range(32):
                nc.tensor.matmul(
                    psw[:, 0:64], lhsT=warm[:, 0:64], rhs=warm[:, 0:64],
                    start=True, stop=True,
                )

            nblk = NBLK * reps
            x8s = {}

            def load_x8(b):
                blk = b % NBLK
                t = xp.tile([128, W], mmdt, tag="x8", name=f"x8_{b}")
                nc.sync.dma_start(
                    out=t[:],
                    in_=x[:, R * blk : R * blk + A, :].bitcast(mmdt),
                )
                x8s[b] = t

            # first input block, then per-dj weight slices, then more input
            # prefetch -- ordered so the first matmul chain's operands arrive
            # as early as possible while the PE warmup is still running.
            load_x8(0)
            for _dj in range(3):
                nc.sync.dma_start(
                    out=lwt[:, _dj, :], in_=lw[_dj].bitcast(mmdt)
                )
            for _pb in range(1, min(3, nblk)):
                load_x8(_pb)

            for b in range(nblk):
                blk = b % NBLK
                r0 = R * blk
                x8 = x8s.pop(b)
                om = outp.tile([128, R, WO], I8, tag="om", name=f"om_{b}")
                om8 = outp.tile([128, WO], I8, tag="om8", name=f"om8_{b}")
                for m in range(NM):
                    pm = psp.tile([128, 512], F32, tag=f"ps{m}", name=f"pm_{b}_{m}")
                    for dj in range(3):
                        nc.tensor.matmul(
                            pm[:, 0:WO],
                            lhsT=lwt[:, dj, 128 * m : 128 * (m + 1)],
                            rhs=x8[:, dj : dj + WO],
                            start=(dj == 0),
                            stop=(dj == 2),
                        )
                    dst = om[:, m, :] if m < R else om8[0:96, :]
                    src = pm[:, 0:WO] if m < R else pm[0:96, 0:WO]
                    if m in DVE_TILES:
                        nc.vector.tensor_scalar(dst, src, 0.0, None, ADD)
                    else:
                        nc.scalar.activation(dst, src, COPY, bias=0.0, scale=1.0)
                if b + 3 < nblk:
                    load_x8(b + 3)
                nc.sync.dma_start(
                    out=y[0:8, :, r0 : r0 + R, :],
                    in_=om[:],
                )
                nc.sync.dma_start(
                    out=y[8, :, r0 : r0 + R, :],
                    in_=om8[0:96, :],
                )
    return nc


def _host_lw(wm):
    """wm = (w*m) [9, 16, 3, 3] fp32 -> lhsT blocks [3, 128, 896].

    Partition row = c*8 + a (a = input-row offset in the 8-row block).
    Chain m in 0..5: col 128m + k*16 + c = wm[k, c, a-m, dj] (patterns 0..7
    of output row r0+m). Chain 6: col 768 + c*6 + r = wm[8, c, a-r, dj]
    (pattern 8 of rows r0+r, r = 0..5)."""
    lw = np.zeros((3, 128, NM * 128), np.float32)
    ks = np.arange(8)
    for dj in range(3):
        for di in range(3):
            for c in range(C):
                for m in range(R):
                    lw[dj, c * 8 + m + di, 128 * m + ks * 16 + c] = wm[:8, c, di, dj]
                for r in range(R):
                    lw[dj, c * 8 + r + di, 768 + c * 6 + r] = wm[8, c, di, dj]
    return lw


def _get_nc(use_f32r=True, reps=1):
    key = ("nc", use_f32r, reps)
    if key not in _CACHE:
        nc_new = _build_nc(use_f32r, reps)
        nc_new.finalize()
        _CACHE[key] = nc_new
    return _CACHE[key]


def _in_maps(xh, xl, wh, wl, mh, ml):
    xh = np.ascontiguousarray(np.asarray(xh, dtype=np.float32))
    xl = np.ascontiguousarray(np.asarray(xl, dtype=np.float32))
    wmh = (np.asarray(wh, np.float32) * np.asarray(mh, np.float32)).astype(np.float32)
    wml = (np.asarray(wl, np.float32) * np.asarray(ml, np.float32)).astype(np.float32)
    maps = []
    for x_all, lw_b in [(xh, _host_lw(wmh)), (xl, _host_lw(wml))]:
        for b in range(B):
            maps.append({"x": np.ascontiguousarray(x_all[b]), "lw": lw_b})
    return maps


def _finish(y_i8):
    return y_i8.astype(np.float32)


def kernel(xh, xl, wh, wl, mh, ml, h=0, use_f32r=True):
    nc = _get_nc(use_f32r)
    in_maps = _in_maps(xh, xl, wh, wl, mh, ml)
    res = run_bass_kernel_spmd(nc, in_maps, list(range(8)))

    out = np.empty((2, 9, B, C, HO, WO), dtype=np.float32)
    for core, rmap in enumerate(res.results):
        br, b = divmod(core, B)
        out[br, :, b] = _finish(rmap["y"])
    return out


def timed_run(xh, xl, wh, wl, mh, ml, h=0, use_f32r=True, iters=5):
    """Returns (out, best_exec_ns): times the sharded PJRT execution with
    device-resident inputs (transfers excluded via pre-device_put)."""
    import jax, time
    from jax.sharding import Mesh, PartitionSpec, NamedSharding
    from concourse import bass2jax, mybir as _mb

    nc = _get_nc(use_f32r)
    in_maps = _in_maps(xh, xl, wh, wl, mh, ml)
    n_cores = 8
    bass2jax.install_neuronx_cc_hook()
    if nc.dbg_addr is not None and not nc.dbg_callbacks:
        in_maps = [
            {**m, nc.dbg_addr.name: np.zeros((1, 2), np.uint32)} for m in in_maps
        ]
    partition_name = nc.partition_id_tensor.name if nc.partition_id_tensor else None
    in_names, out_names, out_avals, zero_outs = [], [], [], []
    for alloc in nc.m.functions[0].allocations:
        if not isinstance(alloc, _mb.MemoryLocationSet):
            continue
        name = alloc.memorylocations[0].name
        if alloc.kind == "ExternalInput":
            if name != partition_name:
                in_names.append(name)
        elif alloc.kind == "ExternalOutput":
            shape = tuple(alloc.tensor_shape)
            dtype = _mb.dt.np(alloc.dtype)
            out_names.append(name)
            out_avals.append(jax.core.ShapedArray(shape, dtype))
            zero_outs.append(np.zeros(shape, dtype))
    n_params = len(in_names)
    n_outs = len(out_avals)
    in_names_all = in_names + out_names
    if partition_name is not None:
        in_names_all.append(partition_name)
    donate = tuple(range(n_params, n_params + n_outs))

    def _body(*args):
        operands = list(args)
        if partition_name is not None:
            operands.append(bass2jax.partition_id_tensor())
        return tuple(
            bass2jax._bass_exec_p.bind(
                *operands,
                out_avals=tuple(out_avals),
                in_names=tuple(in_names_all),
                out_names=tuple(out_names),
                lowering_input_output_aliases=(),
                sim_require_finite=True,
                sim_require_nnan=True,
                nc=nc,
            )
        )

    devices = jax.devices()[:n_cores]
    mesh = Mesh(np.asarray(devices), ("core",))
    from jax.experimental.shard_map import shard_map
    in_specs = (PartitionSpec("core"),) * (n_params + n_outs)
    out_specs = (PartitionSpec("core"),) * n_outs
    sharded = jax.jit(
        shard_map(_body, mesh=mesh, in_specs=in_specs, out_specs=out_specs,
                  check_rep=False),
        donate_argnums=donate, keep_unused=True,
    )
    sh = NamedSharding(mesh, PartitionSpec("core"))
    concat_in = [
        jax.device_put(
            np.concatenate([np.asarray(in_maps[c][nm]) for c in range(n_cores)], axis=0),
            sh,
        )
        for nm in in_names
    ]
    best = None
    out_arrs = None
    for _ in range(max(1, iters)):
        concat_zeros = [
            jax.device_put(np.zeros((n_cores * z.shape[0], *z.shape[1:]), z.dtype), sh)
            for z in zero_outs
        ]
        jax.block_until_ready(concat_zeros)
        t0 = time.perf_counter_ns()
        out_arrs = sharded(*concat_in, *concat_zeros)
        jax.block_until_ready(out_arrs)
        t1 = time.perf_counter_ns()
        if best is None or t1 - t0 < best:
            best = t1 - t0
    out = np.empty((2, 9, B, C, HO, WO), dtype=np.float32)
    arr = np.asarray(out_arrs[0]).reshape(n_cores, 9, C, HO, WO)
    for core in range(n_cores):
        br, b = divmod(core, B)
        out[br, :, b] = _finish(arr[core])
    return out, best


if __name__ == "__main__":
    rng = np.random.RandomState(0)
    ins = {
        "xh": rng.randn(B, C, H, W).astype(np.float32) * 20,
        "xl": rng.randn(B, C, H, W).astype(np.float32) * 20,
        "wh": rng.randn(9, C, 3, 3).astype(np.float32),
        "wl": rng.randn(9, C, 3, 3).astype(np.float32),
        "mh": np.round(rng.rand(9, C, 3, 3)).astype(np.float32),
        "ml": np.round(rng.rand(9, C, 3, 3)).astype(np.float32),
        "h": 0,
    }
    out = kernel(**ins)
    print("kernel out:", out.shape, out.dtype, out.min(), out.max())


# revision 21
# speedup vs baseline: 2.5239x; 1.0916x over previous
"""Trainium2 Bass kernel: 9-pattern masked depthwise 3x3 conv, 2 branches.

Full problem: xh, xl [4, 16, 512, 512] fp32; wh, wl, mh, ml [9, 16, 3, 3].
out = stack([conv9(xh, wh*mh), conv9(xl, wl*ml)])  -> [2, 9, 4, 16, 510, 510]
with clamp(-128, 127) and round-half-even applied elementwise.

Sharding: pure data parallel over (branch, batch) = 8 independent slices,
one per NeuronCore. No cross-core communication.

Per-core kernel strategy (row-blocked matmul + vector-engine assist,
~3.17 matmuls/row):
  - Output rows are processed in 85 blocks of 6. Per block one DMA loads the
    8 input rows it needs as x8[(a,c), w] (partition = a*16+c, a = row
    offset 0..7) -- each input row lands in SBUF once (1.33x total reads).
  - 6 accumulating matmul chains per block (M=128, K=128, N=510) produce
    output row r0+m for patterns 0..7 (M = k*16+c); each chain is 3 fp32r
    matmuls (dj = 0,1,2) whose rhs is the same x8 tile shifted by dj in
    the free dim; di lives in the block-diagonal lhsT (partition a =
    r_off + di). Pattern 8 gets only its dj=0 taps from one extra matmul
    (M = c*8+r); its 6 remaining taps are scalar_tensor_tensor FMAs on
    the otherwise-idle DVE and GPSIMD engines -- in the (c,r) output
    layout the operand partition map is affine (out q=c*8+r reads x8
    partition q+di), so out = x8[q+di, j+dj]*w8[c,di,dj] + acc chains
    directly. 19 matmuls * 510 cycles per block; DVE/GPSIMD split the
    width so every engine finishes inside the PE's block time.
  - Post-processing is ONE instruction per chain: the hardware's fp32 ->
    int8 dtype conversion rounds half-even AND saturates to [-128, 127],
    so a plain Copy activation (ACT) from PSUM to an int8 SBUF tile
    implements round+clamp+convert exactly (the last pattern-8 FMA
    writes int8 directly the same way).
  - int8 results DMA to HBM in (k c) row-major order, 6 contiguous rows
    per partition (3060B descriptors = full DMA bandwidth); the host
    up-converts to fp32 losslessly.
  - fp32r sacrifices ~11 mantissa bits in the matmul operands, flipping
    ~0.4% of outputs by +-1 at round boundaries (rel l2 err ~1.5e-3);
    use_f32r=False gives exact-fp32 at ~4x the time.
"""

import numpy as np

import concourse.bacc as bacc
import concourse.mybir as mybir
from concourse.tile import TileContext
from concourse.bass_utils import run_bass_kernel_spmd

B, C, H, W = 4, 16, 512, 512
HO, WO = H - 2, W - 2
R = 6            # output rows per block
A = R + 2        # input rows per block
NBLK = HO // R   # 85
NM = 7           # matmul chains per block (6 main rows + 1 pattern-8)

MAGIC = 12582912.0  # 1.5 * 2**23 = 192 * 2**16: fp32 RNE round magic
F32 = mybir.dt.float32
F32R = mybir.dt.float32r
I8 = mybir.dt.int8
ADD = mybir.AluOpType.add
COPY = mybir.ActivationFunctionType.Copy

MULT = mybir.AluOpType.mult
CW = 264  # pattern-8 FMA width split: DVE cols [0, CW), GPSIMD [CW, WO)
TAPS = [(di, dj) for dj in (1, 2) for di in (0, 1, 2)]  # engine-side p8 taps

_CACHE = {}


def _build_nc(use_f32r=True, reps=1):
    nc = bacc.Bacc()
    mmdt = F32R if use_f32r else F32

    x = nc.declare_dram_parameter("x", [C, H, W], F32, isOutput=False)
    lw = nc.declare_dram_parameter("lw", [3, 128, NM * 128], F32, isOutput=False)
    w8 = nc.declare_dram_parameter("w8", [128, len(TAPS)], F32, isOutput=False)
    y = nc.declare_dram_parameter("y", [9, C, HO, WO], I8, isOutput=True)

    with TileContext(nc) as tc:
        with (
            tc.tile_pool(name="lwp", bufs=1) as lwp,
            tc.tile_pool(name="xp", bufs=4) as xp,
            tc.tile_pool(name="outp", bufs=2) as outp,
            tc.tile_pool(name="psm", bufs=1, space="PSUM") as psp,
        ):
            lwt = lwp.tile([128, 3, NM * 128], mmdt)

            # PE p-state warmup: the Tensor engine ramps to full clock only
            # after ~3us of continuous execution; these dependency-free dummy
            # matmuls run while the first DMAs are in flight so every real
            # matmul is issued against a fully-ramped engine.
            warm = lwp.tile([128, 64], mmdt, tag="warm")
            nc.vector.memset(warm[:].bitcast(F32), 0)
            psw = psp.tile([64, 64], F32, tag="warm")
            for _i in range(32):
                nc.tensor.matmul(
                    psw[:, 0:64], lhsT=warm[:, 0:64], rhs=warm[:, 0:64],
                    start=True, stop=True,
                )

            nblk = NBLK * reps
            x8s = {}

            def load_x8(b):
                blk = b % NBLK
                t = xp.tile([128, W], F32, tag="x8", name=f"x8_{b}")
                nc.sync.dma_start(
                    out=t[:],
                    in_=x[:, R * blk : R * blk + A, :],
                )
                x8s[b] = t

            # first input block, then per-dj weight slices, then more input
            # prefetch -- ordered so the first matmul chain's operands arrive
            # as early as possible while the PE warmup is still running.
            load_x8(0)
            s6 = lwp.tile([128, len(TAPS)], F32, tag="s6")
            nc.sync.dma_start(out=s6[:], in_=w8[:])
            for _dj in range(3):
                nc.sync.dma_start(
                    out=lwt[:, _dj, :], in_=lw[_dj].bitcast(mmdt)
                )
            for _pb in range(1, min(3, nblk)):
                load_x8(_pb)

            NP8 = C * 8 - 2  # 126: partitions q = c*8 + r (r<6 used)
            for b in range(nblk):
                blk = b % NBLK
                r0 = R * blk
                x8 = x8s.pop(b)
                om = outp.tile([128, R, WO], I8, tag="om", name=f"om_{b}")
                om8 = outp.tile([128, WO], I8, tag="om8", name=f"om8_{b}")

                # pattern-8 dj=0 partial on the PE first, so DVE/GPSIMD can
                # start their FMA chains while the main chains run.
                pm6 = psp.tile([128, 512], F32, tag="ps6", name=f"pm6_{b}")
                nc.tensor.matmul(
                    pm6[:, 0:WO],
                    lhsT=lwt[:, 0, 768:896],
                    rhs=x8[:, 0:WO].bitcast(mmdt),
                    start=True,
                    stop=True,
                )
                accs = {}
                for half, (c0, c1) in (("d", (0, CW)), ("p", (CW, WO))):
                    eng = nc.vector if half == "d" else nc.gpsimd
                    nw = c1 - c0
                    for t, (di, dj) in enumerate(TAPS):
                        last = t == len(TAPS) - 1
                        if last:
                            dst = om8[0:NP8, c0:c1]
                        else:
                            acc = outp.tile(
                                [128, nw], F32, tag=f"acc{half}{t % 2}",
                                name=f"acc_{b}_{half}_{t}",
                            )
                            dst = acc[0:NP8, :]
                        in1 = (
                            pm6[0:NP8, c0:c1] if t == 0 else accs[half][0:NP8, :]
                        )
                        eng.scalar_tensor_tensor(
                            dst,
                            in0=x8[di : di + NP8, c0 + dj : c1 + dj],
                            scalar=s6[0:NP8, t : t + 1],
                            in1=in1,
                            op0=MULT,
                            op1=ADD,
                        )
                        if not last:
                            accs[half] = acc

                for m in range(R):
                    pm = psp.tile([128, 512], F32, tag=f"ps{m}", name=f"pm_{b}_{m}")
                    for dj in range(3):
                        nc.tensor.matmul(
                            pm[:, 0:WO],
                            lhsT=lwt[:, dj, 128 * m : 128 * (m + 1)],
                            rhs=x8[:, dj : dj + WO].bitcast(mmdt),
                            start=(dj == 0),
                            stop=(dj == 2),
                        )
                    nc.scalar.activation(
                        om[:, m, :], pm[:, 0:WO], COPY, bias=0.0, scale=1.0
                    )
                if b + 3 < nblk:
                    load_x8(b + 3)
                nc.sync.dma_start(
                    out=y[0:8, :, r0 : r0 + R, :],
                    in_=om[:],
                )
                nc.sync.dma_start(
                    out=y[8, :, r0 : r0 + R, :],
                    in_=om8[:].rearrange("(c a) w -> c a w", c=C)[:, 0:R, :],
                )
    return nc


def _host_lw(wm):
    """wm = (w*m) [9, 16, 3, 3] fp32 -> lhsT blocks [3, 128, 896].

    Partition row = c*8 + a (a = input-row offset in the 8-row block).
    Chain m in 0..5: col 128m + k*16 + c = wm[k, c, a-m, dj] (patterns 0..7
    of output row r0+m). Chain 6 (dj=0 only): col 768 + c*8 + r =
    wm[8, c, a-r, 0] (pattern 8 of rows r0+r, r = 0..5)."""
    lw = np.zeros((3, 128, NM * 128), np.float32)
    ks = np.arange(8)
    for dj in range(3):
        for di in range(3):
            for c in range(C):
                for m in range(R):
                    lw[dj, c * 8 + m + di, 128 * m + ks * 16 + c] = wm[:8, c, di, dj]
                if dj == 0:
                    for r in range(R):
                        lw[0, c * 8 + r + di, 768 + c * 8 + r] = wm[8, c, di, 0]
    return lw


def _host_w8(wm):
    """Per-partition FMA scalars for the engine-side pattern-8 taps:
    w8[c*8 + r, t] = wm[8, c, di_t, dj_t] (same weight for every row r)."""
    w8 = np.zeros((128, len(TAPS)), np.float32)
    for t, (di, dj) in enumerate(TAPS):
        for c in range(C):
            w8[c * 8 : c * 8 + R, t] = wm[8, c, di, dj]
    return w8


def _get_nc(use_f32r=True, reps=1):
    key = ("nc", use_f32r, reps)
    if key not in _CACHE:
        nc_new = _build_nc(use_f32r, reps)
        nc_new.finalize()
        _CACHE[key] = nc_new
    return _CACHE[key]


def _in_maps(xh, xl, wh, wl, mh, ml):
    xh = np.ascontiguousarray(np.asarray(xh, dtype=np.float32))
    xl = np.ascontiguousarray(np.asarray(xl, dtype=np.float32))
    wmh = (np.asarray(wh, np.float32) * np.asarray(mh, np.float32)).astype(np.float32)
    wml = (np.asarray(wl, np.float32) * np.asarray(ml, np.float32)).astype(np.float32)
    maps = []
    for x_all, wm_b in [(xh, wmh), (xl, wml)]:
        lw_b = _host_lw(wm_b)
        w8_b = _host_w8(wm_b)
        for b in range(B):
            maps.append({"x": np.ascontiguousarray(x_all[b]), "lw": lw_b, "w8": w8_b})
    return maps


def _finish(y_i8):
    return y_i8.astype(np.float32)


def kernel(xh, xl, wh, wl, mh, ml, h=0, use_f32r=True):
    nc = _get_nc(use_f32r)
    in_maps = _in_maps(xh, xl, wh, wl, mh, ml)
    res = run_bass_kernel_spmd(nc, in_maps, list(range(8)))

    out = np.empty((2, 9, B, C, HO, WO), dtype=np.float32)
    for core, rmap in enumerate(res.results):
        br, b = divmod(core, B)
        out[br, :, b] = _finish(rmap["y"])
    return out


def timed_run(xh, xl, wh, wl, mh, ml, h=0, use_f32r=True, iters=5):
    """Returns (out, best_exec_ns): times the sharded PJRT execution with
    device-resident inputs (transfers excluded via pre-device_put)."""
    import jax, time
    from jax.sharding import Mesh, PartitionSpec, NamedSharding
    from concourse import bass2jax, mybir as _mb

    nc = _get_nc(use_f32r)
    in_maps = _in_maps(xh, xl, wh, wl, mh, ml)
    n_cores = 8
    bass2jax.install_neuronx_cc_hook()
    if nc.dbg_addr is not None and not nc.dbg_callbacks:
        in_maps = [
            {**m, nc.dbg_addr.name: np.zeros((1, 2), np.uint32)} for m in in_maps
        ]
    partition_name = nc.partition_id_tensor.name if nc.partition_id_tensor else None
    in_names, out_names, out_avals, zero_outs = [], [], [], []
    for alloc in nc.m.functions[0].allocations:
        if not isinstance(alloc, _mb.MemoryLocationSet):
            continue
        name = alloc.memorylocations[0].name
        if alloc.kind == "ExternalInput":
            if name != partition_name:
                in_names.append(name)
        elif alloc.kind == "ExternalOutput":
            shape = tuple(alloc.tensor_shape)
            dtype = _mb.dt.np(alloc.dtype)
            out_names.append(name)
            out_avals.append(jax.core.ShapedArray(shape, dtype))
            zero_outs.append(np.zeros(shape, dtype))
    n_params = len(in_names)
    n_outs = len(out_avals)
    in_names_all = in_names + out_names
    if partition_name is not None:
        in_names_all.append(partition_name)
    donate = tuple(range(n_params, n_params + n_outs))

    def _body(*args):
        operands = list(args)
        if partition_name is not None:
            operands.append(bass2jax.partition_id_tensor())
        return tuple(
            bass2jax._bass_exec_p.bind(
                *operands,
                out_avals=tuple(out_avals),
                in_names=tuple(in_names_all),
                out_names=tuple(out_names),
                lowering_input_output_aliases=(),
                sim_require_finite=True,
                sim_require_nnan=True,
                nc=nc,
            )
        )

    devices = jax.devices()[:n_cores]
    mesh = Mesh(np.asarray(devices), ("core",))
    from jax.experimental.shard_map import shard_map
    in_specs = (PartitionSpec("core"),) * (n_params + n_outs)
    out_specs = (PartitionSpec("core"),) * n_outs
    sharded = jax.jit(
        shard_map(_body, mesh=mesh, in_specs=in_specs, out_specs=out_specs,
                  check_rep=False),
        donate_argnums=donate, keep_unused=True,
    )
    sh = NamedSharding(mesh, PartitionSpec("core"))
    concat_in = [
        jax.device_put(
            np.concatenate([np.asarray(in_maps[c][nm]) for c in range(n_cores)], axis=0),
            sh,
        )
        for nm in in_names
    ]
    best = None
    out_arrs = None
    for _ in range(max(1, iters)):
        concat_zeros = [
            jax.device_put(np.zeros((n_cores * z.shape[0], *z.shape[1:]), z.dtype), sh)
            for z in zero_outs
        ]
        jax.block_until_ready(concat_zeros)
        t0 = time.perf_counter_ns()
        out_arrs = sharded(*concat_in, *concat_zeros)
        jax.block_until_ready(out_arrs)
        t1 = time.perf_counter_ns()
        if best is None or t1 - t0 < best:
            best = t1 - t0
    out = np.empty((2, 9, B, C, HO, WO), dtype=np.float32)
    arr = np.asarray(out_arrs[0]).reshape(n_cores, 9, C, HO, WO)
    for core in range(n_cores):
        br, b = divmod(core, B)
        out[br, :, b] = _finish(arr[core])
    return out, best


if __name__ == "__main__":
    rng = np.random.RandomState(0)
    ins = {
        "xh": rng.randn(B, C, H, W).astype(np.float32) * 20,
        "xl": rng.randn(B, C, H, W).astype(np.float32) * 20,
        "wh": rng.randn(9, C, 3, 3).astype(np.float32),
        "wl": rng.randn(9, C, 3, 3).astype(np.float32),
        "mh": np.round(rng.rand(9, C, 3, 3)).astype(np.float32),
        "ml": np.round(rng.rand(9, C, 3, 3)).astype(np.float32),
        "h": 0,
    }
    out = kernel(**ins)
    print("kernel out:", out.shape, out.dtype, out.min(), out.max())


# revision 31
# speedup vs baseline: 2.6305x; 1.0422x over previous
"""Trainium2 Bass kernel: 9-pattern masked depthwise 3x3 conv, 2 branches.

Full problem: xh, xl [4, 16, 512, 512] fp32; wh, wl, mh, ml [9, 16, 3, 3].
out = stack([conv9(xh, wh*mh), conv9(xl, wl*ml)])  -> [2, 9, 4, 16, 510, 510]
with clamp(-128, 127) and round-half-even applied elementwise.

Sharding: pure data parallel over (branch, batch) = 8 independent slices,
one per NeuronCore. No cross-core communication.

Per-core kernel strategy (row-blocked matmul + vector-engine assist,
3 matmuls/row):
  - Output rows are processed in 85 blocks of 6. Per block one DMA loads the
    8 input rows it needs as x8[(a,c), w] (partition = a*16+c, a = row
    offset 0..7) -- each input row lands in SBUF once (1.33x total reads).
  - 6 accumulating matmul chains per block (M=128, K=128, N=510) produce
    output row r0+m for patterns 0..7 (M = k*16+c); each chain is 3 fp32r
    matmuls (dj = 0,1,2) whose rhs is the same x8 tile shifted by dj in
    the free dim; di lives in the block-diagonal lhsT (partition a =
    r_off + di). Pattern 8 never touches the PE: its 9 taps run as
    per-partition-scalar FMAs (scalar_tensor_tensor) on the otherwise-
    idle DVE and GPSIMD engines -- in the (c,r) output layout the
    operand partition map is affine (out q=c*8+r reads x8 partition
    q+di), so acc = x8[q+di, j+dj]*w8[c,di,dj] + acc chains directly.
    18 matmuls * 510 cycles per block; DVE/GPSIMD split the width so
    every engine finishes inside the PE's block time.
  - Post-processing is ONE instruction per chain: the hardware's fp32 ->
    int8 dtype conversion rounds half-even AND saturates to [-128, 127],
    so a plain Copy activation (ACT) from PSUM to an int8 SBUF tile
    implements round+clamp+convert exactly (the last pattern-8 FMA
    writes int8 directly the same way).
  - int8 results DMA to HBM in (k c) row-major order, 6 contiguous rows
    per partition (3060B descriptors = full DMA bandwidth); the host
    up-converts to fp32 losslessly.
  - fp32r sacrifices ~11 mantissa bits in the matmul operands, flipping
    ~0.4% of outputs by +-1 at round boundaries (rel l2 err ~1.5e-3);
    use_f32r=False gives exact-fp32 at ~4x the time.
"""

import numpy as np

import concourse.bacc as bacc
import concourse.mybir as mybir
from concourse.tile import TileContext
from concourse.bass_utils import run_bass_kernel_spmd

B, C, H, W = 4, 16, 512, 512
HO, WO = H - 2, W - 2
R = 6            # output rows per block
A = R + 2        # input rows per block
NBLK = HO // R   # 85

MAGIC = 12582912.0  # 1.5 * 2**23 = 192 * 2**16: fp32 RNE round magic
F32 = mybir.dt.float32
F32R = mybir.dt.float32r
I8 = mybir.dt.int8
ADD = mybir.AluOpType.add
COPY = mybir.ActivationFunctionType.Copy

MULT = mybir.AluOpType.mult
CW = 308  # pattern-8 FMA width split: DVE cols [0, CW), GPSIMD [CW, WO)
TAPS = [(di, dj) for dj in (0, 1, 2) for di in (0, 1, 2)]  # all 9 p8 taps

_CACHE = {}


def _build_nc(use_f32r=True, reps=1):
    nc = bacc.Bacc()
    mmdt = F32R if use_f32r else F32

    x = nc.declare_dram_parameter("x", [C, H, W], F32, isOutput=False)
    lw = nc.declare_dram_parameter("lw", [3, 128, R * 128], F32, isOutput=False)
    w8 = nc.declare_dram_parameter("w8", [128, len(TAPS)], F32, isOutput=False)
    y = nc.declare_dram_parameter("y", [9, C, HO, WO], I8, isOutput=True)

    with TileContext(nc) as tc:
        with (
            tc.tile_pool(name="lwp", bufs=1) as lwp,
            tc.tile_pool(name="xp", bufs=4) as xp,
            tc.tile_pool(name="outp", bufs=2) as outp,
            tc.tile_pool(name="psm", bufs=1, space="PSUM") as psp,
        ):
            lwt = lwp.tile([128, 3, R * 128], mmdt)

            # PE p-state warmup: the Tensor engine ramps to full clock only
            # after ~3us of continuous execution; these dependency-free dummy
            # matmuls run while the first DMAs are in flight so every real
            # matmul is issued against a fully-ramped engine.
            warm = lwp.tile([128, 64], mmdt, tag="warm")
            nc.vector.memset(warm[:].bitcast(F32), 0)
            psw = psp.tile([64, 64], F32, tag="warm")
            for _i in range(32):
                nc.tensor.matmul(
                    psw[:, 0:64], lhsT=warm[:, 0:64], rhs=warm[:, 0:64],
                    start=True, stop=True,
                )

            nblk = NBLK * reps
            x8s = {}

            def load_x8(b):
                blk = b % NBLK
                t = xp.tile([128, W], mmdt, tag="x8", name=f"x8_{b}")
                nc.sync.dma_start(
                    out=t[:],
                    in_=x[:, R * blk : R * blk + A, :].bitcast(mmdt),
                )
                x8s[b] = t

            # first input block, then per-dj weight slices, then more input
            # prefetch -- ordered so the first matmul chain's operands arrive
            # as early as possible while the PE warmup is still running.
            load_x8(0)
            s6 = lwp.tile([128, len(TAPS)], F32, tag="s6")
            nc.sync.dma_start(out=s6[:], in_=w8[:])
            for _dj in range(3):
                nc.sync.dma_start(
                    out=lwt[:, _dj, :], in_=lw[_dj].bitcast(mmdt)
                )
            for _pb in range(1, min(3, nblk)):
                load_x8(_pb)

            NP8 = C * 8 - 2  # 126: partitions q = c*8 + r (r<6 used)
            for b in range(nblk):
                blk = b % NBLK
                r0 = R * blk
                x8 = x8s.pop(b)
                om = outp.tile([128, R, WO], I8, tag="om", name=f"om_{b}")
                om8 = outp.tile([128, WO], I8, tag="om8", name=f"om8_{b}")

                # Pattern 8 runs entirely on DVE/GPSIMD as 9 FMA taps (the
                # affine (c,r) layout: out q=c*8+r reads x8 partition q+di),
                # freeing the PE for the 18 main-chain matmuls.
                accs = {}
                for half, (c0, c1) in (("d", (0, CW)), ("p", (CW, WO))):
                    eng = nc.vector if half == "d" else nc.gpsimd
                    nw = c1 - c0
                    for t, (di, dj) in enumerate(TAPS):
                        last = t == len(TAPS) - 1
                        if last:
                            dst = om8[0:NP8, c0:c1]
                        else:
                            acc = outp.tile(
                                [128, nw], F32, tag=f"acc{half}{t % 2}",
                                name=f"acc_{b}_{half}_{t}",
                            )
                            dst = acc[0:NP8, :]
                        in0 = x8[di : di + NP8, c0 + dj : c1 + dj].bitcast(F32)
                        sc = s6[0:NP8, t : t + 1]
                        if t == 0:
                            eng.tensor_scalar(dst, in0, sc, None, MULT)
                        else:
                            eng.scalar_tensor_tensor(
                                dst,
                                in0=in0,
                                scalar=sc,
                                in1=accs[half][0:NP8, :],
                                op0=MULT,
                                op1=ADD,
                            )
                        if not last:
                            accs[half] = acc

                for m in range(R):
                    pm = psp.tile([128, 512], F32, tag=f"ps{m}", name=f"pm_{b}_{m}")
                    for dj in range(3):
                        nc.tensor.matmul(
                            pm[:, 0:WO],
                            lhsT=lwt[:, dj, 128 * m : 128 * (m + 1)],
                            rhs=x8[:, dj : dj + WO],
                            start=(dj == 0),
                            stop=(dj == 2),
                        )
                    nc.scalar.activation(
                        om[:, m, :], pm[:, 0:WO], COPY, bias=0.0, scale=1.0
                    )
                if b + 3 < nblk:
                    load_x8(b + 3)
                nc.sync.dma_start(
                    out=y[0:8, :, r0 : r0 + R, :],
                    in_=om[:],
                )
                nc.sync.dma_start(
                    out=y[8, :, r0 : r0 + R, :],
                    in_=om8[:].rearrange("(c a) w -> c a w", c=C)[:, 0:R, :],
                )
    return nc


def _host_lw(wm):
    """wm = (w*m) [9, 16, 3, 3] fp32 -> lhsT blocks [3, 128, 896].

    Partition row = c*8 + a (a = input-row offset in the 8-row block).
    Chain m in 0..5: col 128m + k*16 + c = wm[k, c, a-m, dj] (patterns 0..7
    of output row r0+m). Pattern 8 is computed on DVE/GPSIMD (see _host_w8)."""
    lw = np.zeros((3, 128, R * 128), np.float32)
    ks = np.arange(8)
    for dj in range(3):
        for di in range(3):
            for c in range(C):
                for m in range(R):
                    lw[dj, c * 8 + m + di, 128 * m + ks * 16 + c] = wm[:8, c, di, dj]
    return lw


def _host_w8(wm):
    """Per-partition FMA scalars for the engine-side pattern-8 taps:
    w8[c*8 + r, t] = wm[8, c, di_t, dj_t] (same weight for every row r)."""
    w8 = np.zeros((128, len(TAPS)), np.float32)
    for t, (di, dj) in enumerate(TAPS):
        for c in range(C):
            w8[c * 8 : c * 8 + R, t] = wm[8, c, di, dj]
    return w8


def _get_nc(use_f32r=True, reps=1):
    key = ("nc", use_f32r, reps)
    if key not in _CACHE:
        nc_new = _build_nc(use_f32r, reps)
        nc_new.finalize()
        _CACHE[key] = nc_new
    return _CACHE[key]


def _in_maps(xh, xl, wh, wl, mh, ml):
    xh = np.ascontiguousarray(np.asarray(xh, dtype=np.float32))
    xl = np.ascontiguousarray(np.asarray(xl, dtype=np.float32))
    wmh = (np.asarray(wh, np.float32) * np.asarray(mh, np.float32)).astype(np.float32)
    wml = (np.asarray(wl, np.float32) * np.asarray(ml, np.float32)).astype(np.float32)
    maps = []
    for x_all, wm_b in [(xh, wmh), (xl, wml)]:
        lw_b = _host_lw(wm_b)
        w8_b = _host_w8(wm_b)
        for b in range(B):
            maps.append({"x": np.ascontiguousarray(x_all[b]), "lw": lw_b, "w8": w8_b})
    return maps


def _finish(y_i8):
    return y_i8.astype(np.float32)


def kernel(xh, xl, wh, wl, mh, ml, h=0, use_f32r=True):
    nc = _get_nc(use_f32r)
    in_maps = _in_maps(xh, xl, wh, wl, mh, ml)
    res = run_bass_kernel_spmd(nc, in_maps, list(range(8)))

    out = np.empty((2, 9, B, C, HO, WO), dtype=np.float32)
    for core, rmap in enumerate(res.results):
        br, b = divmod(core, B)
        out[br, :, b] = _finish(rmap["y"])
    return out


def timed_run(xh, xl, wh, wl, mh, ml, h=0, use_f32r=True, iters=5):
    """Returns (out, best_exec_ns): times the sharded PJRT execution with
    device-resident inputs (transfers excluded via pre-device_put)."""
    import jax, time
    from jax.sharding import Mesh, PartitionSpec, NamedSharding
    from concourse import bass2jax, mybir as _mb

    nc = _get_nc(use_f32r)
    in_maps = _in_maps(xh, xl, wh, wl, mh, ml)
    n_cores = 8
    bass2jax.install_neuronx_cc_hook()
    if nc.dbg_addr is not None and not nc.dbg_callbacks:
        in_maps = [
            {**m, nc.dbg_addr.name: np.zeros((1, 2), np.uint32)} for m in in_maps
        ]
    partition_name = nc.partition_id_tensor.name if nc.partition_id_tensor else None
    in_names, out_names, out_avals, zero_outs = [], [], [], []
    for alloc in nc.m.functions[0].allocations:
        if not isinstance(alloc, _mb.MemoryLocationSet):
            continue
        name = alloc.memorylocations[0].name
        if alloc.kind == "ExternalInput":
            if name != partition_name:
                in_names.append(name)
        elif alloc.kind == "ExternalOutput":
            shape = tuple(alloc.tensor_shape)
            dtype = _mb.dt.np(alloc.dtype)
            out_names.append(name)
            out_avals.append(jax.core.ShapedArray(shape, dtype))
            zero_outs.append(np.zeros(shape, dtype))
    n_params = len(in_names)
    n_outs = len(out_avals)
    in_names_all = in_names + out_names
    if partition_name is not None:
        in_names_all.append(partition_name)
    donate = tuple(range(n_params, n_params + n_outs))

    def _body(*args):
        operands = list(args)
        if partition_name is not None:
            operands.append(bass2jax.partition_id_tensor())
        return tuple(
            bass2jax._bass_exec_p.bind(
                *operands,
                out_avals=tuple(out_avals),
                in_names=tuple(in_names_all),
                out_names=tuple(out_names),
                lowering_input_output_aliases=(),
                sim_require_finite=True,
                sim_require_nnan=True,
                nc=nc,
            )
        )

    devices = jax.devices()[:n_cores]
    mesh = Mesh(np.asarray(devices), ("core",))
    from jax.experimental.shard_map import shard_map
    in_specs = (PartitionSpec("core"),) * (n_params + n_outs)
    out_specs = (PartitionSpec("core"),) * n_outs
    sharded = jax.jit(
        shard_map(_body, mesh=mesh, in_specs=in_specs, out_specs=out_specs,
                  check_rep=False),
        donate_argnums=donate, keep_unused=True,
    )
    sh = NamedSharding(mesh, PartitionSpec("core"))
    concat_in = [
        jax.device_put(
            np.concatenate([np.asarray(in_maps[c][nm]) for c in range(n_cores)], axis=0),
            sh,
        )
        for nm in in_names
    ]
    best = None
    out_arrs = None
    for _ in range(max(1, iters)):
        concat_zeros = [
            jax.device_put(np.zeros((n_cores * z.shape[0], *z.shape[1:]), z.dtype), sh)
            for z in zero_outs
        ]
        jax.block_until_ready(concat_zeros)
        t0 = time.perf_counter_ns()
        out_arrs = sharded(*concat_in, *concat_zeros)
        jax.block_until_ready(out_arrs)
        t1 = time.perf_counter_ns()
        if best is None or t1 - t0 < best:
            best = t1 - t0
    out = np.empty((2, 9, B, C, HO, WO), dtype=np.float32)
    arr = np.asarray(out_arrs[0]).reshape(n_cores, 9, C, HO, WO)
    for core in range(n_cores):
        br, b = divmod(core, B)
        out[br, :, b] = _finish(arr[core])
    return out, best


if __name__ == "__main__":
    rng = np.random.RandomState(0)
    ins = {
        "xh": rng.randn(B, C, H, W).astype(np.float32) * 20,
        "xl": rng.randn(B, C, H, W).astype(np.float32) * 20,
        "wh": rng.randn(9, C, 3, 3).astype(np.float32),
        "wl": rng.randn(9, C, 3, 3).astype(np.float32),
        "mh": np.round(rng.rand(9, C, 3, 3)).astype(np.float32),
        "ml": np.round(rng.rand(9, C, 3, 3)).astype(np.float32),
        "h": 0,
    }
    out = kernel(**ins)
    print("kernel out:", out.shape, out.dtype, out.min(), out.max())
